# revision 21
# baseline (speedup 1.0000x reference)
"""Trainium2 Bass kernel for nn_Attention_11158325035119.

Reference computation (B=2, N=2048, DIM=1024, H=16, DH=64):
  LayerNorm(x) -> Q,K,V projections -> softmax(Q K^T) V (raw logits, no
  1/sqrt(d) scale) -> output projection.

Sharding over 8 NeuronCores: data-parallel on batch (2 groups of 4 cores),
tensor-parallel on heads within each group (4 heads/core, Wq/Wkv
column-sharded).  Each core's normalized attention output is redistributed
with a per-head AllToAll so every core ends up with all heads for a quarter
of its batch's rows and computes a disjoint out-row-slice; the host
concatenates (and transposes) the 8 slices.

The runtime only supports AllToAll on the full 8-core mesh, so each core
duplicates its 4 q-quarter shards into both groups' slots.  On the receive
side the wrong-batch copies are masked out with a per-core 0/1 vector
(gsel) and the two group slots are combined with masked multiply-adds, so
the output projection contracts over the true 1024-dim inner dimension
(no zero-padded doubling) while the program stays SPMD.

Precision: the whole Q/K path (xnT, Wq/Wk, Q^T/K^T, QK^T matmul) is
fp32r — bf16 anywhere on that path pushes the softmax logits (|.|~50)
past the error budget.  V, exp(S), the AllToAll payload, and the output
projection are bf16.

Performance notes (the HAM clock-gate dominates everything):
  - The PE clock throttles to 1.2 GHz after any ~1us idle window and only
    recovers after ~3.4us of dense matmuls.  The kernel therefore keeps
    the PE stream gapless: transposes run in the main matmul stream, the
    Q projections for seq chunks 2-3 are deferred into attention head 0's
    slots as ramp filler across the front->attention boundary, and the
    per-head epilogue runs at high scheduler priority so it never blocks
    the next head's stream.
  - Attention is software-pipelined per (head, q-half): QK^T (2 matmuls)
    -> exp on ScalarE ([128,1024] PSUM->SBUF bf16) -> AV lagged one tile
    so the PE never waits on an exp in flight.  PSUM: 2x s_ps halves
    (4 banks) + 2 O^T accumulators (2 banks) + filler pool (2 banks).
  - O^T_ext = [V|1]^T @ expS (M=65: row 64 accumulates the softmax
    denominators in the matmul).  The denominator row is folded to 8
    partitions by DMA before the (8 cyc/elem, per-lane) DVE reciprocal,
    then broadcast via a 0-stride DRAM read.
  - A first dummy AllToAll absorbs the ~11.5us collective-stream setup.
  - The output projection keeps Wout as the stationary operand (fresh
    Ofull data streams as the moving operand: LDWEIGHTS can be pulled
    ahead of semaphore waits inside the PE reorder window, so stationary
    operands must never be written just before use).  It runs in two
    passes: even inner blocks (heads 0-1) under head 3's AllToAll, odd
    blocks after its arrival, producing out^T.
"""

import numpy as np

import concourse.bass as bass
import concourse.tile as tile
from concourse import mybir
from concourse.masks import make_identity

F32 = mybir.dt.float32
F32R = mybir.dt.float32r
BF16 = mybir.dt.bfloat16

EPS = 1e-5

B, N, DIM = 2, 2048, 1024
H, DH = 16, 64
N_CORES = 8
LANES = 4            # cores per batch group (head-parallel)
HL = H // LANES      # local heads per core


# ---------------------------------------------------------------------------
# Environment workarounds
# ---------------------------------------------------------------------------

def _install_drain_split():
    """walrus in this image rejects InstDrain with >1 sem wait ("Too many
    sync wait commands").  Replace the TileContext tail drain with a chain
    of drains, each waiting on a single proc's semaphore."""
    import re
    import bass_rust

    def _split_drain_and_barrier(self, tick_clock, wait_clock):
        nc = self.nc
        gc = tick_clock.global_clock
        ticks = [int(v) for v in re.findall(r"\d+", repr(gc))]
        for proc, t in [(i, t) for i, t in enumerate(ticks) if t > 0]:
            pc = bass_rust.VectorClock()
            pc.require_at_least(proc, t)
            d = nc.sync.drain()
            wait_clock.add_sem_waits(d.ins, bass_rust.ScopedClock({None: pc}))
        nc.all_engine_barrier()
        assert self.sems is not None
        popped = nc._tile_sem_poison_stack.pop()
        assert popped is self._sem_poison
        nc.clear_and_free_semaphores(list(self.sems.allocated().values()))
        nc.all_engine_barrier()

    tile.TileContext._drain_and_barrier = _split_drain_and_barrier


def _install_profile_shim():
    """Provide antenv.axon_hooks (NTFF profiling via libaxon_pjrt.so) and a
    no-op upload_artifacts (no artifact bucket in this container)."""
    import sys
    import types
    import contextlib
    import ctypes
    import os
    import concourse.bass_utils as bu

    if "antenv.axon_hooks" not in sys.modules:
        hook = None
        so_path = "/opt/axon/libaxon_pjrt.so"
        if os.path.exists(so_path):
            lib = ctypes.CDLL(so_path)
            if hasattr(lib, "axon_start_nrt_profile"):
                lib.axon_start_nrt_profile.argtypes = [
                    ctypes.POINTER(ctypes.c_int64), ctypes.c_size_t]
                lib.axon_start_nrt_profile.restype = ctypes.c_int64
                lib.axon_stop_nrt_profile.argtypes = [ctypes.c_char_p]
                lib.axon_stop_nrt_profile.restype = ctypes.c_int64

                @contextlib.contextmanager
                def _hook(output_dir, device_ids):
                    import jax
                    jax.devices()
                    if device_ids:
                        ids = (ctypes.c_int64 * len(device_ids))(*device_ids)
                        rc = lib.axon_start_nrt_profile(ids, len(device_ids))
                    else:
                        rc = lib.axon_start_nrt_profile(None, 0)
                    if rc != 0:
                        raise RuntimeError(f"axon_start_nrt_profile rc={rc}")
                    try:
                        yield
                    finally:
                        lib.axon_stop_nrt_profile(str(output_dir).encode())
                hook = _hook
        mod = types.ModuleType("antenv.axon_hooks")
        mod.get_axon_ntff_profile_hook = lambda: hook
        mod.set_axon_ntff_profile_hook = lambda h: None
        sys.modules["antenv.axon_hooks"] = mod

    bu.upload_artifacts = lambda tmpdir: f"file://{tmpdir}"


_NOPW = [0]


def split_multi_waits(nc):
    """walrus in this image rejects any engine instruction carrying more
    than one semaphore wait ("Too many sync wait commands").  Hoist extra
    waits onto InstNoOps inserted immediately before the instruction on the
    same engine — semantically identical (the waits are a conjunction and
    execute in stream order)."""
    for f in nc.m.functions:
        for blk in f.blocks:
            il = blk.instructions
            i = 0
            while i < len(il):
                inst = il[i]
                si = inst.sync_info
                if si is not None and si.on_wait is not None \
                        and len(si.on_wait) > 1:
                    waits = list(si.on_wait)
                    inst.sync_info = mybir.SyncInfo(
                        on_wait=[waits[-1]],
                        on_update=list(si.on_update or []))
                    for w in waits[:-1]:
                        _NOPW[0] += 1
                        nop = mybir.InstNoOp(name=f"nopw-{_NOPW[0]}")
                        nop.engine = inst.engine
                        nop.sync_info = mybir.SyncInfo(on_wait=[w],
                                                       on_update=[])
                        il.insert(i, nop)
                        i += 1
                i += 1
    return nc


def _install_neff_cache():
    """Disk-cache walrus NEFF compiles by bir_json content hash (a fresh
    process otherwise pays the full 10-25 min neuronxcc compile every run)."""
    import hashlib
    import os
    import shutil
    import concourse.bass_utils as bu
    import concourse.bass2jax as b2j

    cache_dir = os.environ.get(
        "BASS_NEFF_CACHE_DIR",
        os.path.join(os.path.dirname(os.path.abspath(__file__)), ".neff_cache"))
    os.makedirs(cache_dir, exist_ok=True)
    orig = bu.compile_bir_kernel

    def cached(bir_json, tmpdir, neff_name="file.neff"):
        key = hashlib.sha256(bir_json).hexdigest()[:32]
        hit = os.path.join(cache_dir, key + ".neff")
        dst = os.path.join(tmpdir, neff_name)
        if os.path.exists(hit):
            shutil.copy(hit, dst)
            return dst
        neff = orig(bir_json, tmpdir, neff_name=neff_name)
        try:
            shutil.copy(neff, hit)
        except OSError:
            pass
        return neff

    bu.compile_bir_kernel = cached
    b2j.compile_bir_kernel = cached


_install_drain_split()
_install_profile_shim()
_install_neff_cache()


# ---------------------------------------------------------------------------
# Device program
# ---------------------------------------------------------------------------

def build(nc: bass.Bass):
    """Emit the per-core Tile program (SPMD: cores differ only in data)."""
    P = 128
    S, D = N, DIM
    ST = S // P          # 16 seq tiles
    DT = D // P          # 8 feat tiles
    NQ = S // 512        # 4 q chunks
    HD = HL * DH         # 256 local head cols
    QSL = S // LANES     # 512 output rows per core
    QT = QSL // P        # 4
    GROUPS = [list(range(N_CORES))]

    x_in = nc.dram_tensor("x", [S, D], F32, kind="ExternalInput").ap()
    wq_in = nc.dram_tensor("wq", [D, HD], F32, kind="ExternalInput").ap()
    wk_in = nc.dram_tensor("wk", [D, HD], F32, kind="ExternalInput").ap()
    wv_in = nc.dram_tensor("wv", [D, HD], F32, kind="ExternalInput").ap()
    gamma_in = nc.dram_tensor("gamma", [D], F32, kind="ExternalInput").ap()
    beta_in = nc.dram_tensor("beta", [D], F32, kind="ExternalInput").ap()
    wout_in = nc.dram_tensor("wout", [D, D], BF16, kind="ExternalInput").ap()
    gsel_in = nc.dram_tensor("gsel", [N_CORES], F32,
                             kind="ExternalInput").ap()
    out_dram = nc.dram_tensor("out", [D, QSL], F32,
                              kind="ExternalOutput").ap()
    rden_d = nc.dram_tensor("rden", [HL, 2, S // 2], F32).ap()
    a2a_in = [nc.dram_tensor(f"a2a_in{h}", [N_CORES, DH, QSL], BF16).ap()
              for h in range(HL)]
    a2a_out = [nc.dram_tensor(f"a2a_out{h}", [N_CORES, DH, QSL], BF16).ap()
               for h in range(HL)]

    with tile.TileContext(nc) as tc:
        with (
            tc.tile_pool(name="const", bufs=1) as const,
            tc.tile_pool(name="big", bufs=1) as big,
        ):
            # ---- small constants ----
            gamma_sb = const.tile([P, DT], F32)
            nc.sync.dma_start(out=gamma_sb,
                              in_=gamma_in.rearrange("(o p) -> p o", p=P))
            beta_sb = const.tile([P, DT], F32)
            nc.sync.dma_start(out=beta_sb,
                              in_=beta_in.rearrange("(o p) -> p o", p=P))
            eps_sb = const.tile([P, 1], F32)
            nc.vector.memset(eps_sb, EPS)
            ident = const.tile([P, P], F32)
            make_identity(nc, ident)
            # gsel: [8] 0/1 mask -> broadcast to all 128 partitions
            gsel_sb = const.tile([P, N_CORES], F32)
            nc.sync.dma_start(out=gsel_sb[0:1, :], in_=gsel_in)
            k = 1
            while k < P:
                nc.sync.dma_start(out=gsel_sb[k:min(2 * k, P), :],
                                  in_=gsel_sb[0:min(k, P - k), :])
                k *= 2

            # warm up the collective stream early: the first collective pays
            # ~11.5us of stream setup; absorb it during the front phase.
            ccw_in = nc.dram_tensor("ccw_in", [N_CORES, 32], BF16).ap()
            ccw_out = nc.dram_tensor("ccw_out", [N_CORES, 32], BF16).ap()
            ccw_sb = const.tile([1, N_CORES * 32], BF16)
            nc.vector.memset(ccw_sb, 0.0)
            nc.sync.dma_start(out=ccw_in, in_=ccw_sb)
            nc.gpsimd.collective_compute(
                "AllToAll", mybir.AluOpType.bypass,
                replica_groups=GROUPS, ins=[ccw_in[:]], outs=[ccw_out[:]])

            # ---- activations that live through attention ----
            QT_sb = big.tile([P, HD // P, S], F32R)
            KT_sb = big.tile([P, HD // P, S], F32R)
            V_sb = big.tile([P, ST, HL, DH + 1], BF16)
            nc.vector.memset(V_sb[:, :, :, DH:DH + 1], 1.0)
            # second half of xnT + Wq/Wv live into attention: the V
            # projections of sts 8-15 and Q chunks 2-3 are deferred into
            # head 0's slots as PE ramp filler (keeps the HAM clock up
            # across the front->attention boundary).
            xnT_b = big.tile([P, DT, S // 2], F32R)
            wq_sb = big.tile([P, DT, HD], F32R)
            wv_sb = big.tile([P, DT, HD], F32R)

            # ======== phases 1-3: LayerNorm + transpose + projections ======
            with (
                tc.tile_pool(name="front", bufs=1) as front,
                tc.tile_pool(name="wstage", bufs=1) as wstage,
                tc.tile_pool(name="xp", bufs=5) as xp,
                tc.tile_pool(name="xnp", bufs=5) as xnp,
                tc.tile_pool(name="stats", bufs=4) as stats,
                tc.tile_pool(name="tp", bufs=3, space="PSUM") as tp,
                tc.tile_pool(name="proj", bufs=2, space="PSUM") as proj,
                tc.tile_pool(name="vproj", bufs=2, space="PSUM") as vproj,
            ):
                xnT_a = front.tile([P, DT, S // 2], F32R)

                def xnT(ft, sl):
                    # seq-sliced view across the two xnT halves
                    lo, hi = sl.start, sl.stop
                    if hi <= S // 2:
                        return xnT_a[:, ft, lo:hi]
                    return xnT_b[:, ft, lo - S // 2:hi - S // 2]

                def load_weight(name, src_ap, w):
                    # fp32r matmul operands must be written pre-rounded by a
                    # compute op; DMA output cannot feed an fp32r matmul.
                    stage = wstage.tile([P, DT, HD], F32, tag="wstage",
                                        name=f"stage_{name}")
                    nc.sync.dma_start(
                        out=stage,
                        in_=src_ap.rearrange("(o p) m -> p o m", p=P))
                    nc.vector.tensor_copy(out=w, in_=stage)
                    return w

                wk_sb = front.tile([P, DT, HD], F32R, tag="wk", name="wk")
                load_weight("wq", wq_in, wq_sb)
                load_weight("wk", wk_in, wk_sb)
                load_weight("wv", wv_in, wv_sb)
                def do_proj_chunk(w_sb, dst, nch, pool):
                    # one tensor's T-proj for one 512-col seq chunk (needs
                    # sts 4*nch..4*nch+3 transposed)
                    for pt in range(HD // P):
                        ps = pool.tile([P, 512], F32, tag="proj",
                                       name=f"proj_{dst.name}_{nch}_{pt}")
                        for kt in range(DT):
                            nc.tensor.matmul(
                                ps, w_sb[:, kt, pt * P:(pt + 1) * P],
                                xnT(kt, slice(nch * 512, (nch + 1) * 512)),
                                start=(kt == 0), stop=(kt == DT - 1))
                        nc.vector.tensor_copy(
                            out=dst[:, pt, nch * 512:(nch + 1) * 512],
                            in_=ps)

                def do_v_st(st, pool):
                    ps = pool.tile([P, HD], F32, tag="vproj",
                                   name=f"vproj_{st}")
                    for kt in range(DT):
                        nc.tensor.matmul(
                            ps, xnT(kt, slice(st * P, (st + 1) * P)),
                            wv_sb[:, kt, :],
                            start=(kt == 0), stop=(kt == DT - 1))
                    nc.vector.tensor_copy(
                        out=V_sb[:, st, :, 0:DH],
                        in_=ps.rearrange("p (h d) -> p h d", h=HL))

                for stg in range(ST // 4):
                    sts = range(4 * stg, 4 * stg + 4)
                    xn_ts = []
                    for st in sts:
                        x_t = xp.tile([P, D], F32, tag="x",
                                      name=f"x_{st}")
                        nc.sync.dma_start(out=x_t,
                                          in_=x_in[st * P:(st + 1) * P, :])
                        stt = stats.tile([P, 2, 6], F32, tag="stt")
                        nc.vector.bn_stats(out=stt[:, 0], in_=x_t[:, :D // 2])
                        nc.vector.bn_stats(out=stt[:, 1], in_=x_t[:, D // 2:])
                        mv = stats.tile([P, 2], F32, tag="mv")
                        nc.vector.bn_aggr(out=mv, in_=stt)
                        std = stats.tile([P, 1], F32, tag="std")
                        nc.scalar.activation(
                            out=std, in_=mv[:, 1:2],
                            func=mybir.ActivationFunctionType.Sqrt,
                            bias=eps_sb)
                        rstd = stats.tile([P, 1], F32, tag="rstd")
                        nc.vector.reciprocal(out=rstd, in_=std)
                        nmr = stats.tile([P, 1], F32, tag="nmr")
                        nc.vector.tensor_scalar(
                            out=nmr, in0=mv[:, 0:1], scalar1=rstd,
                            scalar2=-1.0,
                            op0=mybir.AluOpType.mult,
                            op1=mybir.AluOpType.mult)
                        # normalize on ScalarE: xn = (x - mu) * rstd
                        xn_t = xnp.tile([P, D], F32, tag="xn",
                                        name=f"xn_{st}")
                        nc.scalar.activation(
                            out=xn_t, in_=x_t,
                            func=mybir.ActivationFunctionType.Identity,
                            scale=rstd, bias=nmr)
                        xn_ts.append(xn_t)
                    # transpose 4 sts per ft into one [128,512] PSUM bank;
                    # single gamma/beta affine copy per bank (ScalarE)
                    for ft in range(DT):
                        pt_ps = tp.tile([P, 4 * P], F32, tag="tp")
                        for si in range(4):
                            nc.tensor.transpose(
                                pt_ps[:, si * P:(si + 1) * P],
                                xn_ts[si][:, ft * P:(ft + 1) * P], ident)
                        dst = xnT(ft, slice(stg * 512, (stg + 1) * 512))
                        if ft % 2 == 0:
                            nc.scalar.activation(
                                out=dst, in_=pt_ps,
                                func=mybir.ActivationFunctionType.Identity,
                                scale=gamma_sb[:, ft:ft + 1],
                                bias=beta_sb[:, ft:ft + 1])
                        else:
                            nc.vector.tensor_scalar(
                                out=dst, in0=pt_ps,
                                scalar1=gamma_sb[:, ft:ft + 1],
                                scalar2=beta_sb[:, ft:ft + 1],
                                op0=mybir.AluOpType.mult,
                                op1=mybir.AluOpType.add)
                    # V projection inline (stationary operand of AV: must
                    # be written well outside the PE reorder window)
                    for st in sts:
                        do_v_st(st, vproj)
                    # K for this chunk always; Q only for chunks 0-1
                    do_proj_chunk(wk_sb, KT_sb, stg, proj)
                    if stg < 2:
                        do_proj_chunk(wq_sb, QT_sb, stg, proj)

            # deferred projections -> PE ramp fillers for attention
            # (ordered so V(st) lands before AV slot st, Q chunks before
            # head 0's qh=1)
            fillers = [
                lambda pool: do_proj_chunk(wq_sb, QT_sb, 2, pool),
                lambda pool: do_proj_chunk(wq_sb, QT_sb, 3, pool),
            ]

            # ======== phases 4-7: attention + AllToAll + out-proj ======
            with tc.tile_pool(name="late", bufs=1) as late:
                wout_sb = late.tile([P, DT, D], BF16)
                nc.sync.dma_start(
                    out=wout_sb,
                    in_=wout_in.rearrange("(o p) m -> p o m", p=P))
                Ofull = late.tile([P, 2 * QT, QSL], BF16)
                out_acc = late.tile([P, QT, D], F32)
                attention_and_outproj(
                    nc, tc, QT_sb, KT_sb, V_sb, Ofull, out_acc, wout_sb,
                    gsel_sb, a2a_in, a2a_out, out_dram, GROUPS, rden_d,
                    fillers)

    return nc


def attention_and_outproj(nc, tc, QT_sb, KT_sb, V_sb, Ofull, out_acc,
                          wout_sb, gsel_sb, a2a_in, a2a_out, out_dram,
                          GROUPS, rden_d, fillers):
    P = 128
    S, D = N, DIM
    ST = S // P
    NQ = S // 512
    QSL = S // LANES
    QT = QSL // P
    HQ = S // 2          # q columns per half
    with (
        tc.tile_pool(name="expp", bufs=3) as expp,
        tc.tile_pool(name="osbp", bufs=2) as osbp,
        tc.tile_pool(name="recp", bufs=2) as recp,
        tc.tile_pool(name="obfp", bufs=2) as obfp,
        tc.tile_pool(name="stgp", bufs=2) as stgp,
        tc.tile_pool(name="spsum", bufs=2, space="PSUM") as spsum,
        tc.tile_pool(name="opsum", bufs=2, space="PSUM") as opsum,
        tc.tile_pool(name="fillp", bufs=1, space="PSUM") as fillp,
    ):
        fillers = list(fillers)
        for h in range(HL):
            kb = (h * DH) % P
            kpt = (h * DH) // P
            o_sb = osbp.tile([DH + 1, S], F32, tag="osum",
                             name=f"o_sb_{h}")
            obf_h = obfp.tile([DH, S], BF16, tag="obf", name=f"obf_{h}")
            for qh in range(2):
                o_ps = [opsum.tile([DH + 1, 512], F32, tag="o",
                                   name=f"o_ps_{h}_{qh}_{i}")
                        for i in range(2)]
                pend = []
                def flush_av():
                    e_t, pt_ = pend.pop(0)
                    for cc in range(2):
                        nc.tensor.matmul(
                            o_ps[cc], V_sb[:, pt_, h, :],
                            e_t[:, cc * 512:(cc + 1) * 512],
                            start=(pt_ == 0), stop=(pt_ == ST - 1))
                for t in range(ST):
                    s_ps = spsum.tile([P, HQ], F32, tag="s",
                                      name=f"s_ps_{h}_{qh}_{t}")
                    for cc in range(2):
                        c = 2 * qh + cc
                        nc.tensor.matmul(
                            s_ps[:, cc * 512:(cc + 1) * 512],
                            KT_sb[kb:kb + DH, kpt, t * P:(t + 1) * P],
                            QT_sb[kb:kb + DH, kpt, c * 512:(c + 1) * 512],
                            start=True, stop=True)
                    e_t = expp.tile([P, HQ], BF16, tag="e",
                                    name=f"e_t_{h}_{qh}_{t}")
                    nc.scalar.activation(
                        out=e_t, in_=s_ps,
                        func=mybir.ActivationFunctionType.Exp)
                    pend.append((e_t, t))
                    if len(pend) > 1:
                        flush_av()
                    if fillers:
                        fillers.pop(0)(fillp)
                while pend:
                    flush_av()

                # per-half tail: drain o_ps, reciprocal of the denominator
                # row, gpsimd partition-broadcast, normalize, stage the two
                # q-quarters of this half for the AllToAll.  High priority:
                # the scheduler must slot these at head completion, ahead
                # of the next head's attention stream.
                with tc.high_priority():
                    qs = slice(qh * HQ, (qh + 1) * HQ)
                    for cc in range(2):
                        nc.vector.tensor_copy(
                            out=o_sb[:, (2 * qh + cc) * 512:
                                     (2 * qh + cc + 1) * 512],
                            in_=o_ps[cc])
                    # reciprocal of the denominator row: fold to 8
                    # partitions first (DVE reciprocal is ~8 cyc/elem on a
                    # single lane otherwise), then broadcast via a 0-stride
                    # DRAM read.
                    fold = recp.tile([8, HQ // 8], F32, tag="fold",
                                     name=f"fold_{h}_{qh}")
                    nc.sync.dma_start(out=fold, in_=o_sb[DH:DH + 1, qs])
                    nc.vector.reciprocal(out=fold, in_=fold)
                    rec_b = recp.tile([DH, HQ], F32, tag="rb",
                                      name=f"rec_b_{h}_{qh}")
                    nc.sync.dma_start(out=rden_d[h, qh], in_=fold)
                    nc.sync.dma_start(
                        out=rec_b, in_=rden_d[h, qh].partition_broadcast(DH))
                    nc.vector.tensor_mul(
                        out=obf_h[:, qs], in0=o_sb[0:DH, qs], in1=rec_b)
                    # quarters 2qh, 2qh+1 -> slots {g*4+2qh, g*4+2qh+1}
                    for g in range(2):
                        nc.sync.dma_start(
                            out=a2a_in[h][g * LANES + 2 * qh:
                                          g * LANES + 2 * qh + 2]
                            .rearrange("j p q -> p j q"),
                            in_=obf_h[:, qs].rearrange(
                                "p (j q) -> p j q", j=2))

            with tc.high_priority():
                nc.gpsimd.collective_compute(
                    "AllToAll", mybir.AluOpType.bypass,
                    replica_groups=GROUPS,
                    ins=[a2a_in[h][:]], outs=[a2a_out[h][:]])
                # gather all 8 slots; combine the two group slots with the
                # 0/1 gsel mask (receive-side batch selection)
                pb = (h % 2) * DH
                stage = stgp.tile([P, N_CORES, QSL], BF16, tag="stg",
                                  name=f"stage_{h}")
                for i in range(N_CORES):
                    nc.gpsimd.dma_start(
                        out=stage[pb:pb + DH, i], in_=a2a_out[h][i])
                for j in range(LANES):
                    tmp = stgp.tile([P, QSL], BF16, tag="tmp",
                                    name=f"tmp_{h}_{j}")
                    nc.vector.tensor_scalar_mul(
                        out=tmp[pb:pb + DH], in0=stage[pb:pb + DH, j],
                        scalar1=gsel_sb[pb:pb + DH, j:j + 1])
                    nc.vector.scalar_tensor_tensor(
                        out=Ofull[pb:pb + DH, 2 * j + h // 2],
                        in0=stage[pb:pb + DH, j + LANES],
                        scalar=gsel_sb[pb:pb + DH,
                                       j + LANES:j + LANES + 1],
                        in1=tmp[pb:pb + DH],
                        op0=mybir.AluOpType.mult,
                        op1=mybir.AluOpType.add)

    # ======== phase 7: output projection (two passes) ========
    # pass 1 (even inner blocks = heads 0,1) runs while head 3's
    # AllToAll is in flight; pass 2 (odd blocks) after its arrival.
    # lhsT is Wout (loaded long ago: safe against LDWEIGHTS pull-ahead);
    # the freshly written Ofull streams as the moving operand.  The
    # result is out^T [dim, q]; the host transposes the slice.
    with (
        tc.tile_pool(name="outp", bufs=2) as outp,
        tc.tile_pool(name="oproj", bufs=4, space="PSUM") as oproj,
    ):
        for dt_ in range(D // P):
            ps = oproj.tile([P, QSL], F32, tag="op1",
                            name=f"op1_{dt_}")
            for kb_i, b in enumerate(range(0, 2 * QT, 2)):
                nc.tensor.matmul(
                    ps, wout_sb[:, b, dt_ * P:(dt_ + 1) * P],
                    Ofull[:, b, :],
                    start=(kb_i == 0), stop=(kb_i == QT - 1))
            nc.vector.tensor_copy(
                out=out_acc[:, dt_ % QT,
                            (dt_ // QT) * QSL:(dt_ // QT + 1) * QSL],
                in_=ps)
        for dt_ in range(D // P):
            ps = oproj.tile([P, QSL], F32, tag="op2",
                            name=f"op2_{dt_}")
            for kb_i, b in enumerate(range(1, 2 * QT, 2)):
                nc.tensor.matmul(
                    ps, wout_sb[:, b, dt_ * P:(dt_ + 1) * P],
                    Ofull[:, b, :],
                    start=(kb_i == 0), stop=(kb_i == QT - 1))
            ot = outp.tile([P, QSL], F32, tag="ot", name=f"ot_{dt_}")
            nc.vector.tensor_add(
                out=ot, in0=ps,
                in1=out_acc[:, dt_ % QT,
                            (dt_ // QT) * QSL:(dt_ // QT + 1) * QSL])
            nc.sync.dma_start(
                out=out_dram[dt_ * P:(dt_ + 1) * P, :], in_=ot)

    return nc


# ---------------------------------------------------------------------------
# Host entry point
# ---------------------------------------------------------------------------

_CACHE = {}


def _get_program():
    key = "v2"
    if key not in _CACHE:
        nc = bass.Bass("TRN2", target_bir_lowering=False, debug=False,
                       num_devices=N_CORES)
        build(nc)
        split_multi_waits(nc)
        _CACHE[key] = nc
    return _CACHE[key]


def _shard_inputs(x, gamma, beta, Wq, Wkv, Wout):
    import ml_dtypes
    x = np.asarray(x, dtype=np.float32)
    gamma = np.ascontiguousarray(np.asarray(gamma, dtype=np.float32))
    beta = np.ascontiguousarray(np.asarray(beta, dtype=np.float32))
    Wq = np.asarray(Wq, dtype=np.float32)
    Wkv = np.asarray(Wkv, dtype=np.float32)
    Wk, Wv = Wkv[:, :H * DH], Wkv[:, H * DH:]
    wout_bf = np.ascontiguousarray(np.asarray(Wout, np.float32)).astype(
        ml_dtypes.bfloat16)
    in_maps = []
    for core in range(N_CORES):
        b = core // LANES
        lane = core % LANES
        cs = slice(lane * HL * DH, (lane + 1) * HL * DH)
        gsel = np.zeros((N_CORES,), dtype=np.float32)
        gsel[b * LANES:(b + 1) * LANES] = 1.0
        m = {
            "x": np.ascontiguousarray(x[b]),
            "wq": np.ascontiguousarray(Wq[:, cs]),
            "wk": np.ascontiguousarray(Wk[:, cs]),
            "wv": np.ascontiguousarray(Wv[:, cs]),
            "gamma": gamma,
            "beta": beta,
            "wout": wout_bf,
            "gsel": gsel,
        }
        in_maps.append(m)
    return in_maps


def _unshard_output(results):
    out = np.empty((B, N, DIM), dtype=np.float32)
    qsl = N // LANES
    for core in range(N_CORES):
        b = core // LANES
        lane = core % LANES
        out[b, lane * qsl:(lane + 1) * qsl, :] = results[core]["out"].T
    return out


def kernel(x, gamma, beta, Wq, Wkv, Wout, trace=False):
    from concourse.bass_utils import run_bass_kernel_spmd
    nc = _get_program()
    in_maps = _shard_inputs(x, gamma, beta, Wq, Wkv, Wout)
    res = run_bass_kernel_spmd(nc, in_maps, list(range(N_CORES)), trace=trace)
    out = _unshard_output(res.results)
    if trace:
        kernel.last_exec_time_ns = res.exec_time_ns
        kernel.last_result = res
    return out
